# revision 5
# baseline (speedup 1.0000x reference)
"""KNN interaction graph (k=32, cutoff=10) on 8 Trainium2 NeuronCores.

Algorithm
---------
reference: full [N,N] masked pairwise-distance matrix + row-wise top-32.
Since ``batch`` is sorted, the masked distance matrix is block-diagonal:
row i's candidates are exactly its own molecule's atoms.  Each core owns
1024 rows (8 tiles of 128); for a 128-row tile every candidate column
lies in the window [mol_start(first row), mol_end(last row)) whose span
is bounded (~264 for these inputs), so one small matmul per tile
replaces a [128, 8192] sweep.

The matmul computes, in one K=9 fp32 contraction (accumulated in this
row order so the penalty terms cancel exactly in f32 before the small
distance terms enter):

  psum[i,j] = PEN*(s_i-s_j)^2 - 2*pos_i.pos_j + sqn_j,   s = (batch-64)/8

then v = -psum - sqn_i = -(d^2) - PEN*(ds)^2.  Cross-molecule pairs get
v <= -256 and clamp to exactly -100 (= -CUTOFF^2), matching the
reference's CUTOFF masking in the negated-square domain.  The diagonal
(v ~ 0) is the strict row max, so a 5-round max8/max_index8/
match_replace top-40 with slot 0 dropped yields the top-32 neighbours in
ascending-distance order with jax.lax.top_k's smallest-index tie-break
(max_index matches first occurrences in index order).  Weights are
sqrt(-v) on ACT (exact for f32).
"""
import os
import numpy as np

try:
    import concourse  # noqa: F401
except ImportError:
    import sys
    sys.path.insert(0, "/opt/trn_rl_repo")

N = 8192
K = 32
P = 128
NCORES = 8
TILES = 8            # 128-row tiles per core
RPC = P * TILES      # rows per core
KR = 9               # augmented contraction depth
PEN = 16384.0
NEG_BIG = -1e30
CUT2 = 100.0         # CUTOFF^2
SENT_SQN = 4.0e4     # sentinel sq-norm for window padding columns

LAST_EXEC_NS = None  # filled when BASS_KNN_TRACE=1

_prog_cache = {}


def _bf16_round(x):
    v = x.astype(np.float32).view(np.uint32)
    return (((v.astype(np.uint64) + 0x8000) & 0xFFFF0000).astype(np.uint32)
            .view(np.float32))


def _build_program(wmax):
    import concourse.tile as tile
    from concourse import bacc, mybir

    nc = bacc.Bacc("TRN2", target_bir_lowering=False)
    f32, i32, u32 = mybir.dt.float32, mybir.dt.int32, mybir.dt.uint32

    a_d = nc.dram_tensor("a_slab", [KR, RPC], f32, kind="ExternalInput")
    b_d = nc.dram_tensor("b_win", [KR, TILES, wmax], f32, kind="ExternalInput")
    nsq_d = nc.dram_tensor("negsqn", [P, TILES], f32, kind="ExternalInput")
    w0_d = nc.dram_tensor("w0", [P, TILES], f32, kind="ExternalInput")
    outw_d = nc.dram_tensor("outw", [TILES, P, K], f32, kind="ExternalOutput")
    outi_d = nc.dram_tensor("outi", [TILES, P, K], i32, kind="ExternalOutput")

    with tile.TileContext(nc) as tc:
        with tc.tile_pool(name="const", bufs=1) as const, \
             tc.tile_pool(name="work", bufs=3) as work, \
             tc.tile_pool(name="sel", bufs=2) as sel, \
             tc.tile_pool(name="ps", bufs=2, space="PSUM") as pp:
            a_s = const.tile([KR, RPC], f32)
            b_s = const.tile([KR, TILES, wmax], f32)
            nsq_s = const.tile([P, TILES], f32)
            w0_s = const.tile([P, TILES], f32)
            nc.gpsimd.dma_start(a_s, a_d[:, :])
            nc.gpsimd.dma_start(b_s, b_d[:, :, :])
            nc.gpsimd.dma_start(nsq_s, nsq_d[:, :])
            nc.gpsimd.dma_start(w0_s, w0_d[:, :])

            for t in range(TILES):
                psum = pp.tile([P, wmax], f32)
                nc.tensor.matmul(psum, a_s[:, t * P:(t + 1) * P], b_s[:, t, :],
                                 start=True, stop=True)
                v = work.tile([P, wmax], f32)
                nc.scalar.activation(v, psum, mybir.ActivationFunctionType.Identity,
                                     bias=nsq_s[:, t:t + 1], scale=-1.0)
                nc.vector.tensor_scalar(v, v, 0.0, -CUT2,
                                        mybir.AluOpType.min, mybir.AluOpType.max)

                vals = sel.tile([P, 40], f32, tag="vals")
                idx = sel.tile([P, 40], u32, tag="idx")
                for r in range(5):
                    mv = vals[:, r * 8:(r + 1) * 8]
                    nc.vector.max(mv, v)
                    nc.vector.max_index(idx[:, r * 8:(r + 1) * 8], mv, v)
                    if r < 4:
                        nc.vector.match_replace(v, mv, v, NEG_BIG)

                gidx_f = sel.tile([P, K], f32, tag="gidxf")
                nc.vector.tensor_scalar(gidx_f, idx[:, 1:K + 1],
                                        w0_s[:, t:t + 1], None,
                                        mybir.AluOpType.add)
                gidx = sel.tile([P, K], i32, tag="gidx")
                nc.vector.tensor_copy(gidx, gidx_f)
                wout = sel.tile([P, K], f32, tag="wout")
                nc.scalar.activation(wout, vals[:, 1:K + 1],
                                     mybir.ActivationFunctionType.Sqrt,
                                     bias=0.0, scale=-1.0)
                nc.sync.dma_start(outi_d[t, :, :], gidx)
                nc.sync.dma_start(outw_d[t, :, :], wout)
    nc.compile()
    return nc


def kernel(pos, batch):
    global LAST_EXEC_NS
    from concourse.bass_utils import run_bass_kernel_spmd

    pos = np.ascontiguousarray(np.asarray(pos), dtype=np.float32)
    b64 = np.asarray(batch).astype(np.int64)
    assert pos.shape == (N, 3) and b64.shape == (N,)

    x, y, z = pos[:, 0], pos[:, 1], pos[:, 2]
    sqn = ((x * x + y * y) + z * z).astype(np.float32)
    s = ((b64 - 64).astype(np.float32)) / np.float32(8.0)
    s2 = s * s
    s2h = _bf16_round(s2)
    s2l = (s2 - s2h).astype(np.float32)
    ones = np.ones(N, np.float32)

    A = np.stack([s2h, s2l, s, ones, ones, x, y, z, ones]).astype(np.float32)
    Bm = np.stack([PEN * ones, PEN * ones, np.float32(-2 * PEN) * s,
                   PEN * s2h, PEN * s2l,
                   np.float32(-2.0) * x, np.float32(-2.0) * y,
                   np.float32(-2.0) * z, sqn]).astype(np.float32)

    # per-tile candidate windows (batch is sorted)
    mol_start = np.searchsorted(b64, b64, side="left")
    mol_end = np.searchsorted(b64, b64, side="right")
    first = np.arange(0, N, P)
    w0g = mol_start[first].astype(np.int64)
    w1g = mol_end[first + P - 1].astype(np.int64)
    span = int((w1g - w0g).max())
    wmax = max(64, (span + 15) // 16 * 16)

    # pad columns with sentinels so every window is exactly wmax wide
    sent = np.zeros((KR, wmax), np.float32)
    sent[8, :] = SENT_SQN
    Bp = np.concatenate([Bm, sent], axis=1)

    in_maps = []
    for c in range(NCORES):
        r0 = c * RPC
        bwin = np.empty((KR, TILES, wmax), np.float32)
        w0c = np.empty(TILES, np.int32)
        for t in range(TILES):
            g = c * TILES + t
            w0c[t] = w0g[g]
            bwin[:, t, :] = Bp[:, w0g[g]:w0g[g] + wmax]
        in_maps.append({
            "a_slab": np.ascontiguousarray(A[:, r0:r0 + RPC]),
            "b_win": bwin,
            "negsqn": np.ascontiguousarray((-sqn[r0:r0 + RPC]).reshape(TILES, P).T),
            "w0": np.ascontiguousarray(
                np.broadcast_to(w0c[None, :].astype(np.float32), (P, TILES))),
        })

    if wmax not in _prog_cache:
        _prog_cache[wmax] = _build_program(wmax)
    nc = _prog_cache[wmax]

    trace = os.environ.get("BASS_KNN_TRACE", "") == "1"
    res = run_bass_kernel_spmd(nc, in_maps, core_ids=list(range(NCORES)),
                               trace=trace)
    LAST_EXEC_NS = res.exec_time_ns

    iw = np.concatenate([r["outw"].reshape(RPC, K) for r in res.results])
    ii = np.concatenate([r["outi"].reshape(RPC, K) for r in res.results])
    edge_index = np.stack([ii.reshape(-1).astype(np.int32),
                           np.repeat(np.arange(N, dtype=np.int32), K)])
    edge_weight = iw.reshape(-1).astype(np.float32)
    return edge_index, edge_weight
